# revision 47
# baseline (speedup 1.0000x reference)
"""Chamfer distance loss kernel for Trainium2 (8 NeuronCores).

Problem: template/source (4, 8192, 3) f32. For each batch b:
  d[n,m] = |t_n|^2 - 2 t_n.s_m + |s_m|^2
  loss_b = mean_n min_m d + mean_m min_n d ; output = mean_b loss_b (scalar).

Sharding: core c handles (batch = c//2, template-row-half = c%2):
4096 template rows x all 8192 source points per core; the 8 per-core
partials are combined on the host (the cross-core reduction is a handful
of tiny vectors).

Per core, each [128 rows x 512 cols] distance tile is produced directly
in PSUM by ONE augmented matmul: d = |t|^2 - 2 t.s + |s|^2 becomes a
K=24 contraction of bf16 3-way value splits (a = a1+a2+a3 exactly in
bf16 parts; products kept to O(2^-27)), giving fp32-grade distances at
1 cycle/row — 4x faster than native fp32 matmuls. ScalarE evacuates
PSUM to SBUF as fp16 (quantizing d to fp16 before the min is safe: the
final rel err stays at ~4e-5), VectorE runs all min-accumulation in
fp16 at its 2x packed mode: a per-row-tile running min over column
groups (folded 2048->512 with 2x tensor_tensor ops before the 1x-only
tensor_reduce), and a per-column running min over row tiles kept in two
half-range accumulators so the first half's output DMA overlaps the
second half's compute. The 128-partition fold of the column minima and
all tiny means/sums happen on the host.
"""
import os
import sys

sys.path.insert(0, "/opt/trn_rl_repo")

from contextlib import ExitStack

import numpy as np

import concourse.bass as bass
import concourse.tile as tile
from concourse import mybir
from concourse.bass_utils import run_bass_kernel_spmd

# ---------------------------------------------------------------------------
# The walrus build in this container rejects instructions carrying more than
# one sync-wait command. After Tile scheduling, split any multi-wait
# instruction: keep the first wait on it and hoist the rest onto standalone
# EventSemaphore instructions inserted just before it (same engine, so
# per-engine program order makes the waits execute first).
import bass_rust as _br


def split_multi_waits(nc):
    n_new = 0
    for fn in nc.m.functions:
        for blk in fn.blocks:
            insts = list(blk.instructions)
            out = []
            changed = False
            for inst in insts:
                si = inst.sync_info
                waits = list(si.on_wait) if si is not None and si.on_wait else []
                if len(waits) > 1:
                    for w in waits[:-1]:
                        ev = _br.InstEventSemaphore(
                            name=f"I-waitsplit-{n_new}", ins=[], outs=[]
                        )
                        n_new += 1
                        ev.engine = inst.engine
                        ev.sync_info = _br.SyncInfo(on_wait=[w], on_update=[])
                        out.append(ev)
                    si.on_wait = [waits[-1]]
                    changed = True
                out.append(inst)
            if changed:
                blk.instructions = out
# ---------------------------------------------------------------------------

import ml_dtypes

F32 = mybir.dt.float32
F32R = mybir.dt.float32r
F16 = mybir.dt.float16
BF16 = mybir.dt.bfloat16
MIN = mybir.AluOpType.min
BF16NP = ml_dtypes.bfloat16

B, N, M, D = 4, 8192, 8192, 3
R = N // 2      # template rows per core
NCORES = 8
GROUP = 2048    # psum group: 4 matmuls of 512

# "dekker": bf16 3-way-split matmuls, K=24 (1 cycle/row; error at the fp16
#           cast floor ~3.7e-5 final rel err — same as exact-fp32 matmuls)
# "f32"   : exact fp32 matmuls, K=5 (4 cycles/row, slowest, exact)
# "f32r"  : float32r matmuls, K=5 (fast but ~tf32 precision: too coarse)
MM_MODE = os.environ.get("CHAMFER_MM_MODE", "dekker")
K_BY_MODE = {"dekker": 24, "f32": 5, "f32r": 5}
K = K_BY_MODE[MM_MODE]

# offload every Nth eligible colacc min to the DMA compute-copy engine
# (gpsimd SWDGE accum: out = min(in, out)); 0 disables
DMAMIN_MOD = int(os.environ.get("CHAMFER_DMAMIN_MOD", "0"))



def build_program(rows=R, cols=M, mm_mode=MM_MODE, split_waits=True):
    row_tiles = rows // 128
    ngroups = cols // GROUP
    k = K_BY_MODE[mm_mode]
    nc = bass.Bass("TRN2", target_bir_lowering=False, debug=False)
    mm_dt = {"dekker": BF16, "f32": F32, "f32r": F32R}[mm_mode]
    lhsT = nc.dram_tensor("lhsT_aug", [k, rows], mm_dt, kind="ExternalInput").ap()
    rhs = nc.dram_tensor("rhs_aug", [k, cols], mm_dt, kind="ExternalInput").ap()
    o_rm = nc.dram_tensor(
        "out_rowmin", [128, row_tiles], F32, kind="ExternalOutput"
    ).ap()
    o_cm = nc.dram_tensor(
        "out_colmin", [2, 128, cols], F16, kind="ExternalOutput"
    ).ap()

    def vmin(out_ap, a_ap, b_ap):
        nc.vector.tensor_tensor(out_ap, a_ap, b_ap, op=MIN)

    with tile.TileContext(nc) as tc, ExitStack() as ctx:
        consts = ctx.enter_context(tc.tile_pool(name="consts", bufs=1))
        psum_pool = ctx.enter_context(tc.tile_pool(name="psum", bufs=2, space="PSUM"))
        cast_pool = ctx.enter_context(tc.tile_pool(name="cast", bufs=6))
        rowacc_pool = ctx.enter_context(tc.tile_pool(name="rowacc", bufs=4))
        rfold_pool = ctx.enter_context(tc.tile_pool(name="rfold", bufs=2))
        accs = ctx.enter_context(tc.tile_pool(name="accs", bufs=1))

        # warm the ACT function-table (its ~2.7us load overlaps input DMA)
        warm = consts.tile([1, 16], F16)
        nc.vector.memset(warm[:], 0.0)
        nc.scalar.copy(warm[:], warm[:])

        lhsT_sb = consts.tile([k, rows], mm_dt)
        nc.sync.dma_start(lhsT_sb[:], lhsT)
        rhs_sb = consts.tile([k, cols], mm_dt)
        # chunked so the first matmuls only wait on their own column range
        for q in range(ngroups):
            nc.sync.dma_start(
                rhs_sb[:, q * GROUP:(q + 1) * GROUP],
                rhs[:, q * GROUP:(q + 1) * GROUP],
            )

        # two colacc halves: the first half's accumulator is final midway
        # through the loop, so its output DMA overlaps the second half
        colaccA = accs.tile([128, cols], F16)
        colaccB = accs.tile([128, cols], F16)
        rowminb = accs.tile([128, row_tiles], F32)
        half_tiles = max(row_tiles // 2, 1)

        for i in range(row_tiles):
            lh = lhsT_sb[:, i * 128:(i + 1) * 128]
            cacc = colaccA if i < half_tiles else colaccB
            first = i == 0 or i == half_tiles
            rowacc = rowacc_pool.tile([128, GROUP], F16)
            for g in range(ngroups):
                ps = psum_pool.tile([128, GROUP], F32)
                for jj in range(4):
                    c0 = g * GROUP + jj * 512
                    nc.tensor.matmul(
                        ps[:, jj * 512:(jj + 1) * 512], lh,
                        rhs_sb[:, c0:c0 + 512],
                        start=True, stop=True,
                    )
                ca = cacc[:, g * GROUP:(g + 1) * GROUP]
                if first:
                    # first row tile of a half initializes colacc from ACT
                    if i == 0 and g == 0:
                        # lead-in: cast in 1024-col halves; bank-level psum
                        # deps let the first cast start after 2 of 4 matmuls
                        for h in (0, 1):
                            sub = slice(h * 1024, (h + 1) * 1024)
                            nc.scalar.copy(ca[:, sub], ps[:, sub])
                            nc.vector.tensor_copy(rowacc[:, sub], ca[:, sub])
                        continue
                    nc.scalar.copy(ca, ps[:])
                    if g == 0:
                        nc.vector.tensor_copy(rowacc[:], ca)
                    else:
                        vmin(rowacc[:], ca, rowacc[:])
                elif g == 0:
                    # ACT writes rowacc directly; colacc mins against it
                    nc.scalar.copy(rowacc[:], ps[:])
                    vmin(ca, rowacc[:], ca)
                else:
                    cst = cast_pool.tile([128, GROUP], F16)
                    nc.scalar.copy(cst[:], ps[:])
                    vmin(rowacc[:], cst[:], rowacc[:])
                    if DMAMIN_MOD and (i * ngroups + g) % DMAMIN_MOD == 0:
                        nc.gpsimd.dma_start(ca, cst[:], accum_op=MIN)
                    else:
                        vmin(ca, cst[:], ca)
                if i == row_tiles - 1 and row_tiles > 1:
                    # this column group of colaccB is final: ship it now so
                    # only the last chunk's DMA is exposed at the tail
                    nc.sync.dma_start(
                        o_cm[1][:, g * GROUP:(g + 1) * GROUP], ca
                    )
            # pre-fold with 2x tt ops before the 1x-only reduce; fold into a
            # separate tile so rowacc's buffer is released after one op
            rfold = rfold_pool.tile([128, 1024], F16)
            vmin(rfold[:], rowacc[:, 0:1024], rowacc[:, 1024:2048])
            vmin(rfold[:, 0:512], rfold[:, 0:512], rfold[:, 512:1024])
            nc.vector.tensor_reduce(
                rowminb[:, i:i + 1], rfold[:, 0:512],
                axis=mybir.AxisListType.X, op=MIN,
            )
            if i == half_tiles - 1 and row_tiles > 1:
                # colaccA final: ship it while the second half computes
                nc.sync.dma_start(o_cm[0], colaccA[:])

        if row_tiles == 1:
            nc.sync.dma_start(o_cm[0], colaccA[:])
            nc.sync.dma_start(o_cm[1], colaccA[:])
        nc.sync.dma_start(o_rm, rowminb[:])
    if split_waits:
        split_multi_waits(nc)  # CoreSim can't model the injected waits
    return nc


_program_cache = {}


def _get_program():
    key = (R, M, MM_MODE)
    if key not in _program_cache:
        _program_cache[key] = build_program()
    return _program_cache[key]


def enable_profiling():
    """Wire up the NTFF profiling hook (the image's antenv lacks
    antenv.axon_hooks) and neuter the credential-requiring artifact upload.
    Needed only when tracing (BASS_TRACE=1); harmless otherwise."""
    import types
    import antenv
    import concourse.bass_utils as _bu

    if "antenv.axon_hooks" not in sys.modules:
        hooks = types.ModuleType("antenv.axon_hooks")
        hooks._h = None
        hooks.set_axon_ntff_profile_hook = lambda h: setattr(hooks, "_h", h)
        hooks.get_axon_ntff_profile_hook = lambda: hooks._h
        sys.modules["antenv.axon_hooks"] = hooks
        antenv.axon_hooks = hooks
        try:
            from trn_agent_boot.trn_boot import _ntff_profile_via_ctypes

            hooks.set_axon_ntff_profile_hook(
                _ntff_profile_via_ctypes("/opt/axon/libaxon_pjrt.so")
            )
        except Exception:
            pass
    _bu.upload_artifacts = lambda tmpdir: f"local:{tmpdir}"


if os.environ.get("BASS_TRACE"):
    try:
        enable_profiling()
    except Exception:
        pass


def _aug_f32(t, s):
    """K=5 fp32 augmentation: d = |t|^2 - 2 t.s + |s|^2 in one matmul."""
    rows, cols = t.shape[0], s.shape[0]
    lhsT = np.empty((5, rows), np.float32)
    lhsT[0:3] = t.T
    lhsT[3] = (t * t).sum(axis=1)
    lhsT[4] = 1.0
    rhs = np.empty((5, cols), np.float32)
    rhs[0:3] = -2.0 * s.T
    rhs[3] = 1.0
    rhs[4] = (s * s).sum(axis=1)
    return lhsT, rhs


def _split3(x):
    x1 = x.astype(BF16NP)
    r = x - x1.astype(np.float32)
    x2 = r.astype(BF16NP)
    x3 = (r - x2.astype(np.float32)).astype(BF16NP)
    return x1, x2, x3


def _aug_dekker(t, s):
    """K=24 bf16 3-way-split augmentation. Each fp32 value a = a1+a2+a3 in
    bf16 parts; products kept to O(2^-27): a1b1, a1b2, a2b1, a1b3, a3b1,
    a2b2. PE cost is free-dim cycles only, so K=24 runs as fast as K=5."""
    rows, cols = t.shape[0], s.shape[0]
    t1, t2, t3 = _split3(t)
    s1, s2, s3 = _split3(-2.0 * s)
    n1, n2, n3 = _split3((t * t).sum(axis=1))
    m1, m2, m3 = _split3((s * s).sum(axis=1))
    one = np.ones((), BF16NP)
    lhsT = np.empty((24, rows), BF16NP)
    for j, part in enumerate((t1, t1, t2, t1, t3, t2)):
        lhsT[3 * j:3 * j + 3] = part.T
    lhsT[18] = n1
    lhsT[19] = n2
    lhsT[20] = n3
    lhsT[21:24] = one
    rhs = np.empty((24, cols), BF16NP)
    for j, part in enumerate((s1, s2, s1, s3, s1, s2)):
        rhs[3 * j:3 * j + 3] = part.T
    rhs[18:21] = one
    rhs[21] = m1
    rhs[22] = m2
    rhs[23] = m3
    return lhsT, rhs


def make_in_maps(template, source, mm_mode=MM_MODE):
    template = np.asarray(template, dtype=np.float32)
    source = np.asarray(source, dtype=np.float32)
    aug = _aug_dekker if mm_mode == "dekker" else _aug_f32
    in_maps = []
    for c in range(NCORES):
        b, h = c // 2, c % 2
        t = template[b, h * R:(h + 1) * R]      # [R, 3]
        s = source[b]                            # [M, 3]
        lhsT, rhs = aug(t, s)
        in_maps.append(
            {"lhsT_aug": np.ascontiguousarray(lhsT),
             "rhs_aug": np.ascontiguousarray(rhs)}
        )
    return in_maps


last_results = None  # BassKernelResults of the most recent kernel() call


def kernel(template, source):
    global last_results
    nc = _get_program()
    in_maps = make_in_maps(template, source)
    res = run_bass_kernel_spmd(nc, in_maps, list(range(NCORES)))
    last_results = res

    per_batch = np.zeros(B, dtype=np.float64)
    for b in range(B):
        r0 = res.results[2 * b + 0]
        r1 = res.results[2 * b + 1]
        rowsum = (
            r0["out_rowmin"].astype(np.float64).sum()
            + r1["out_rowmin"].astype(np.float64).sum()
        )
        cost_p0_p1 = rowsum / N
        cm = np.minimum(
            r0["out_colmin"].astype(np.float32).reshape(-1, M).min(axis=0),
            r1["out_colmin"].astype(np.float32).reshape(-1, M).min(axis=0),
        )
        cost_p1_p0 = cm.astype(np.float64).mean()
        per_batch[b] = cost_p0_p1 + cost_p1_p0
    return np.float32(per_batch.mean())
